# revision 28
# baseline (speedup 1.0000x reference)
# Dense GAT layer (4 heads, dim 64) on Trainium2 via Bass/Tile.
#
# Math: h = x@W; e_ij = LeakyReLU(src_i + dst_j, 0.2); masked softmax over j
# with valid = adj & mask_i & mask_j; out = LN((alpha @ h) * mask_i).
#
# Key ideas:
#   * Host-side node compaction: rows/cols with mask==0 contribute nothing
#     (their output is just beta); gather the valid nodes on host, run the
#     kernel on the compacted [m, m] problem (padded to a multiple of 128),
#     scatter back.  Cuts n^2 elementwise work ~2.5x and all DMA traffic.
#   * exp(LeakyReLU(t)) = max(exp(t), exp(0.2 t)),  t = src_i + dst_j
#     exp(src_i + dst_j) = exp(src_i) * exp(dst_j)   (rank-1 separable)
#   * "e^T" orientation: j (softmax axis) on partitions, i on the free axis,
#     so alpha@h needs no transposes and rowsum is a matmul ones-column.
#   * Per-head route split: some heads compute exp(Prelu(t)) on the ACT
#     engine (replicated src row in PSUM + per-partition dst bias), others
#     use the separable max form on the DVE.  adj-mask multiply placement is
#     tunable between DVE and GPSIMD.
#   * rstd for LayerNorm = exp(-0.5*ln(var+eps)) so every ACT function used
#     (exp/parametric_relu/ln/copy) lives in ONE activation table set
#     (natural_log_exp_and_others) -> no ~2.7us table reloads.
#   * adjT/xT pre-transposed on host -> no on-device DMA transposes.
# Sharding: data-parallel, 2 graphs per core across 8 cores.

import numpy as np

H, D = 4, 64
HD = H * D
EPS = 1e-5
NCORES = 8

_PROG_CACHE = {}

# Tuning knobs (baked into the compiled program; cache key includes them).
CFG = dict(
    na=(5, 5, 0, 0),        # per head: number of j-chunks on the ACT route
    head_order=(0, 2, 1, 3),
    mask_gps=(),            # adj-mul on GPSIMD ((h,jc) pairs)
    osb_act=(0, 1),         # heads whose 1/rowsum scale runs on ACT
    rep_cast_act=True,      # arep/crep PSUM->f16 casts on ACT (else DVE)
    hones_act=False,        # hones copy on ACT (else DVE)
    b4op=True,              # B route: ts+ts+max instead of ts+stt
)


def _build_program(ng, NV, KC, trivial_ln, cfg):
    import concourse.bacc as bacc
    import concourse.mybir as mybir
    import concourse.tile as tile
    from concourse.bass import ts

    f16 = mybir.dt.float16
    f32 = mybir.dt.float32
    AF = mybir.ActivationFunctionType
    OP = mybir.AluOpType

    n_v = NV * 128
    E = D + 1               # head block in hones (64 h cols + 1 ones col)
    in_dim = KC * 128

    na = cfg["na"]
    head_order = cfg["head_order"]
    mask_gps = set(cfg["mask_gps"])
    osb_act = set(cfg["osb_act"])
    # matmul moving-operand pieces (<=512 psum f32 columns per instruction)
    pieces = [(s, min(512, n_v - s)) for s in range(0, n_v, 512)]

    nc = bacc.Bacc()

    xT = nc.dram_tensor("xT", [ng, in_dim, n_v], f16, kind="ExternalInput")
    adjT = nc.dram_tensor("adjT", [ng, n_v, n_v], f16, kind="ExternalInput")
    wc = nc.dram_tensor("wc", [128, KC * (HD + H)], f16, kind="ExternalInput")
    wsd = nc.dram_tensor("wsd", [128, KC * H], f16, kind="ExternalInput")
    ones16 = nc.dram_tensor("ones16", [1, 128], f16, kind="ExternalInput")
    if not trivial_ln:
        gam = nc.dram_tensor("gamma_rep", [128, HD], f32, kind="ExternalInput")
        bet = nc.dram_tensor("beta_rep", [128, HD], f32, kind="ExternalInput")
    out = nc.dram_tensor("out", [ng, n_v, HD], f16, kind="ExternalOutput")

    from contextlib import ExitStack

    with tile.TileContext(nc) as tc, ExitStack() as ctx:
        def pool(**kw):
            return ctx.enter_context(tc.tile_pool(**kw))

        consts = pool(name="consts", bufs=1)
        xt_pool = pool(name="xt", bufs=2 * KC)
        adjt_pool = pool(name="adjt", bufs=NV + 2)
        rows_pool = pool(name="rows", bufs=3)
        flat_pool = pool(name="flat", bufs=2)
        reps_pool = pool(name="reps", bufs=3)
        hones_pool = pool(name="hones", bufs=NV + 2)
        small_pool = pool(name="small", bufs=2 * (NV + 2))
        ew_pool = pool(name="ew", bufs=4)
        u_pool = pool(name="u", bufs=2 * NV + 2)
        osb_pool = pool(name="osb", bufs=NV + 2)
        ln_pool = pool(name="ln", bufs=4)
        misc_pool = pool(name="misc", bufs=3)
        # PSUM (8 banks of 2KB): pbig 1x2 + ph 1x1 + pav 5x1
        pbig_pool = pool(name="pbig", bufs=1, space="PSUM")
        ph_pool = pool(name="ph", bufs=1, space="PSUM")
        pav_pool = pool(name="pav", bufs=NV, space="PSUM")

        # ---- pin the ACT spline table set that covers Exp/Prelu/Ln/Copy,
        # so the compiler never inserts mid-kernel ~1.3us table reloads ----
        from concourse.hw_specs import get_activation_tables

        tabs = list(get_activation_tables(nc.m.arch).items())
        set_id = next(
            i for i, (nm, fs) in enumerate(tabs)
            if {AF.Exp, AF.Prelu, AF.Ln, AF.Copy} <= fs
        )
        nc.scalar.add_instruction(
            mybir.InstLoadActFuncSet(
                name=nc.get_next_instruction_name(),
                act_func_set_id=set_id,
                ins=[],
                outs=[],
            )
        )

        # ---- constants ----
        ones_sb = consts.tile([1, 128], f16, tag="ones")
        nc.sync.dma_start(ones_sb[:], ones16[:])
        wc_sb = consts.tile([128, KC * (HD + H)], f16, tag="wc")
        nc.sync.dma_start(wc_sb[:], wc[:])
        wsd_sb = consts.tile([128, KC * H], f16, tag="wsd")
        nc.sync.dma_start(wsd_sb[:], wsd[:])
        if not trivial_ln:
            gam_sb = consts.tile([128, HD], f32, tag="gam")
            nc.sync.dma_start(gam_sb[:], gam[:])
            bet_sb = consts.tile([128, HD], f32, tag="bet")
            nc.sync.dma_start(bet_sb[:], bet[:])
        eps_sb = consts.tile([128, 1], f32, tag="eps")
        nc.vector.memset(eps_sb[:], EPS)

        need_a = any(na[h] > 0 for h in range(H))
        need_b = any(na[h] < NV for h in range(H))

        for g in range(ng):
            # ---- input DMAs (host pre-transposed; plain loads) ----
            xt = []
            for kc in range(KC):
                t = xt_pool.tile([128, n_v], f16, tag="xt")
                nc.sync.dma_start(t[:], xT[g, ts(kc, 128)])
                xt.append(t)
            adjt = []
            for jc in range(NV):
                t = adjt_pool.tile([128, n_v], f16, tag="adjt")
                nc.sync.dma_start(t[:], adjT[g, ts(jc, 128)])
                adjt.append(t)

            # ---- src rows: psd[h, i] = (x @ Wa_src)^T ----
            psd = pbig_pool.tile([H, n_v], f32, tag="pbig")
            for s, w_ in pieces:
                for kc in range(KC):
                    nc.tensor.matmul(
                        psd[:, s : s + w_],
                        wsd_sb[:, ts(kc, H)],
                        xt[kc][:, s : s + w_],
                        start=(kc == 0),
                        stop=(kc == KC - 1),
                    )
            if need_b:
                arow = rows_pool.tile([H, n_v], f16, tag="arow")
                nc.scalar.activation(arow[:], psd[:], AF.Exp)
                crow = rows_pool.tile([H, n_v], f16, tag="crow")
                nc.scalar.activation(crow[:], psd[:], AF.Exp, scale=0.2)
                arowx = flat_pool.tile([1, H * n_v], f16, tag="arowx")
                nc.sync.dma_start(
                    arowx[:].rearrange("p (h w) -> p h w", h=H), arow[:]
                )
                crowx = flat_pool.tile([1, H * n_v], f16, tag="crowx")
                nc.sync.dma_start(
                    crowx[:].rearrange("p (h w) -> p h w", h=H), crow[:]
                )
            if need_a:
                srow = rows_pool.tile([H, n_v], f16, tag="srow")
                nc.scalar.copy(srow[:], psd[:])
                srowx = flat_pool.tile([1, H * n_v], f16, tag="srowx")
                nc.sync.dma_start(
                    srowx[:].rearrange("p (h w) -> p h w", h=H), srow[:]
                )

            # ---- replicate rows for all B-route heads first, so the DVE can
            # start the separable route while ACT runs the Prelu/Exp route ----
            mv8 = ln_pool.tile([128, 2 * NV], f32, tag="mv8", name=f"mv8_{g}")
            areps = {}
            creps = {}
            for h in head_order:
                if na[h] >= NV:
                    continue
                pr = pbig_pool.tile([128, n_v], f32, tag="pbig")
                for s, w_ in pieces:
                    nc.tensor.matmul(
                        pr[:, s : s + w_],
                        ones_sb[:],
                        arowx[0:1, h * n_v + s : h * n_v + s + w_],
                        start=True,
                        stop=True,
                    )
                arep = reps_pool.tile([128, n_v], f16, tag="arep")
                if cfg["rep_cast_act"]:
                    nc.scalar.copy(arep[:], pr[:])
                else:
                    nc.vector.tensor_copy(arep[:], pr[:])
                areps[h] = arep
                pr2 = pbig_pool.tile([128, n_v], f32, tag="pbig")
                for s, w_ in pieces:
                    nc.tensor.matmul(
                        pr2[:, s : s + w_],
                        ones_sb[:],
                        crowx[0:1, h * n_v + s : h * n_v + s + w_],
                        start=True,
                        stop=True,
                    )
                crep = reps_pool.tile([128, n_v], f16, tag="crep")
                if cfg["rep_cast_act"]:
                    nc.scalar.copy(crep[:], pr2[:])
                else:
                    nc.vector.tensor_copy(crep[:], pr2[:])
                creps[h] = crep

            # ---- h_ext per chunk: hones (fp16 h + ones col), dst scalars ----
            # (after the rep matmuls so the serialized ph ring does not
            # head-of-line-block the PE queue for the replicates)
            hones = []
            Bm = []
            Dm = []
            dcol = []
            for ic in range(NV):
                ph = ph_pool.tile([128, HD + H], f32, tag="ph")
                for kc in range(KC):
                    nc.tensor.matmul(
                        ph[:],
                        xt[kc][:, ts(ic, 128)],
                        wc_sb[:, ts(kc, HD + H)],
                        start=(kc == 0),
                        stop=(kc == KC - 1),
                    )
                # dst columns to SBUF first (ph is freed by two readers)
                dc = small_pool.tile([128, H], f32, tag="dcol")
                nc.vector.tensor_copy(dc[:], ph[:, HD : HD + H])
                dcol.append(dc)
                ho = hones_pool.tile([128, H * E], f16, tag="hones")
                ho3 = ho[:].rearrange("p (h e) -> p h e", h=H)
                if cfg["hones_act"]:
                    nc.scalar.copy(
                        ho3[:, :, 0:D],
                        ph[:, 0:HD].rearrange("p (h d) -> p h d", h=H),
                    )
                else:
                    nc.vector.tensor_copy(
                        ho3[:, :, 0:D],
                        ph[:, 0:HD].rearrange("p (h d) -> p h d", h=H),
                    )
                nc.vector.memset(ho3[:, :, D : D + 1], 1.0)
                hones.append(ho)
                if need_b:
                    bm = small_pool.tile([128, H], f32, tag="bm")
                    nc.scalar.activation(bm[:], dc[:], AF.Exp)
                    Bm.append(bm)
                    dm = small_pool.tile([128, H], f32, tag="dm")
                    nc.scalar.activation(dm[:], dc[:], AF.Exp, scale=0.2)
                    Dm.append(dm)
                else:
                    Bm.append(None)
                    Dm.append(None)

            # ---- elementwise u tiles + per-head alpha@h accumulation ----
            o_sb = [
                osb_pool.tile([128, HD], f32, tag="osb", name=f"osb_{g}_{i}")
                for i in range(NV)
            ]
            pav = [
                pav_pool.tile([128, H * E], f32, tag="pav", name=f"pav_{g}_{i}")
                for i in range(NV)
            ]
            for h in head_order:
                nah = na[h]
                a_jcs = list(range(nah))
                b_jcs = list(range(nah, NV))

                srep = None
                if a_jcs:
                    srep = pbig_pool.tile([128, n_v], f32, tag="pbig")
                    for s, w_ in pieces:
                        nc.tensor.matmul(
                            srep[:, s : s + w_],
                            ones_sb[:],
                            srowx[0:1, h * n_v + s : h * n_v + s + w_],
                            start=True,
                            stop=True,
                        )

                u_tiles = [None] * NV
                for jc in a_jcs:
                    lrt = ew_pool.tile([128, n_v], f16, tag="lrt")
                    nc.scalar.activation(
                        lrt[:], srep[:], AF.Prelu,
                        bias=dcol[jc][:, h : h + 1], alpha=0.2,
                    )
                    up = ew_pool.tile([128, n_v], f16, tag="up")
                    nc.scalar.activation(up[:], lrt[:], AF.Exp)
                    u = u_pool.tile([128, n_v], f16, tag="u")
                    meng = nc.gpsimd if (h, jc) in mask_gps else nc.vector
                    meng.tensor_mul(u[:], up[:], adjt[jc][:])
                    u_tiles[jc] = u
                for jc in b_jcs:
                    arep, crep = areps[h], creps[h]
                    t2 = ew_pool.tile([128, n_v], f16, tag="t2")
                    nc.vector.tensor_scalar(
                        t2[:], crep[:], Dm[jc][:, h : h + 1], None, op0=OP.mult
                    )
                    w = ew_pool.tile([128, n_v], f16, tag="w")
                    if cfg["b4op"]:
                        t1 = ew_pool.tile([128, n_v], f16, tag="t1")
                        nc.vector.tensor_scalar(
                            t1[:], arep[:], Bm[jc][:, h : h + 1], None,
                            op0=OP.mult,
                        )
                        nc.vector.tensor_max(w[:], t1[:], t2[:])
                    else:
                        nc.vector.scalar_tensor_tensor(
                            w[:], arep[:], Bm[jc][:, h : h + 1], t2[:],
                            op0=OP.mult, op1=OP.max,
                        )
                    u = u_pool.tile([128, n_v], f16, tag="u")
                    meng = nc.gpsimd if (h, jc) in mask_gps else nc.vector
                    meng.tensor_mul(u[:], w[:], adjt[jc][:])
                    u_tiles[jc] = u

                # alpha@h: this head's block of every chunk's PSUM tile, so
                # the PE works during the elementwise phase instead of after
                for ic in range(NV):
                    for jc in range(NV):
                        nc.tensor.matmul(
                            pav[ic][:, ts(h, E)],
                            u_tiles[jc][:, ts(ic, 128)],
                            hones[jc][:, ts(h, E)],
                            start=(jc == 0),
                            stop=(jc == NV - 1),
                        )

            for ic in range(NV):
                pav3 = pav[ic][:].rearrange("p (h e) -> p h e", h=H)
                rs4 = ln_pool.tile([128, H], f32, tag="rs4")
                nc.vector.reciprocal(rs4[:], pav3[:, :, D])
                for hh in range(H):
                    if hh in osb_act:
                        nc.scalar.mul(
                            o_sb[ic][:, ts(hh, D)],
                            pav3[:, hh, 0:D],
                            rs4[:, hh : hh + 1],
                        )
                    else:
                        nc.vector.tensor_scalar(
                            o_sb[ic][:, ts(hh, D)],
                            pav3[:, hh, 0:D],
                            rs4[:, hh : hh + 1],
                            None,
                            op0=OP.mult,
                        )
                st6 = ln_pool.tile([128, 6], f32, tag="st6")
                nc.vector.bn_stats(st6[:], o_sb[ic][:])
                nc.vector.bn_aggr(mv8[:, 2 * ic : 2 * ic + 2], st6[:])

            # ---- LayerNorm apply + output ----
            # rstd = exp(-0.5 * ln(var + eps)); ln/exp share the table set
            # with Prelu/Exp above (natural_log_exp_and_others).
            mvv = mv8[:].rearrange("p (c two) -> p c two", two=2)
            lnv = ln_pool.tile([128, NV], f32, tag="lnv")
            nc.scalar.activation(lnv[:], mvv[:, :, 1], AF.Ln, bias=eps_sb[:])
            rstd = ln_pool.tile([128, NV], f32, tag="rstd")
            nc.scalar.activation(rstd[:], lnv[:], AF.Exp, scale=-0.5)
            # apply on ACT (idle at the tail): (o - mu)*rstd = o*rstd + nmr
            nmr = ln_pool.tile([128, NV], f32, tag="nmr")
            nc.vector.scalar_tensor_tensor(
                nmr[:], mvv[:, :, 0], -1.0, rstd[:], op0=OP.mult, op1=OP.mult
            )
            for ic in range(NV):
                o2 = misc_pool.tile([128, HD], f16, tag="o2")
                if trivial_ln:
                    nc.scalar.activation(
                        o2[:],
                        o_sb[ic][:],
                        AF.Identity,
                        bias=nmr[:, ic : ic + 1],
                        scale=rstd[:, ic : ic + 1],
                    )
                else:
                    o3 = misc_pool.tile([128, HD], f32, tag="o3")
                    nc.vector.tensor_scalar(
                        o3[:],
                        o_sb[ic][:],
                        mv8[:, 2 * ic : 2 * ic + 1],
                        rstd[:, ic : ic + 1],
                        op0=OP.subtract,
                        op1=OP.mult,
                    )
                    nc.vector.tensor_mul(o3[:], o3[:], gam_sb[:])
                    nc.vector.tensor_add(o2[:], o3[:], bet_sb[:])
                nc.gpsimd.dma_start(out[g, ts(ic, 128), :], o2[:])

    nc.compile()
    return nc


def _host_prep(x, adj, mask, W, a_src, a_dst, gamma, beta, ng, NV, idxs):
    """Per-core input maps: compaction + dtype packing + weight folding."""
    b, n, in_dim = x.shape
    KC = in_dim // 128
    n_v = NV * 128

    # Fold attention vectors into W:  Wa[c, h] = sum_d W[c, h*D+d] * a[h, d]
    Wr = W.astype(np.float64).reshape(in_dim, H, D)
    wa_src = np.einsum("chd,hd->ch", Wr, a_src.astype(np.float64))
    wa_dst = np.einsum("chd,hd->ch", Wr, a_dst.astype(np.float64))

    wc_full = np.ascontiguousarray(
        np.concatenate(
            [W.astype(np.float16), wa_dst.astype(np.float16)], axis=1
        )
        .reshape(KC, 128, HD + H)
        .transpose(1, 0, 2)
    ).reshape(128, KC * (HD + H))
    wsd_full = np.ascontiguousarray(
        wa_src.astype(np.float16).reshape(KC, 128, H).transpose(1, 0, 2)
    ).reshape(128, KC * H)
    ones16 = np.ones((1, 128), np.float16)

    x16 = x.astype(np.float16)
    adj01 = adj != 0

    in_maps = []
    for c in range(NCORES):
        xT = np.zeros((ng, in_dim, n_v), np.float16)
        adjTc = np.zeros((ng, n_v, n_v), np.float16)
        for gl in range(ng):
            g = c * ng + gl
            idx = idxs[g]
            m = len(idx)
            xT[gl, :, :m] = x16[g][idx].T
            adjTc[gl, :m, :m] = adj01[g][np.ix_(idx, idx)].T
        m_map = {
            "xT": xT,
            "adjT": adjTc,
            "wc": wc_full,
            "wsd": wsd_full,
            "ones16": ones16,
        }
        if not (np.all(gamma == 1.0) and np.all(beta == 0.0)):
            m_map["gamma_rep"] = np.ascontiguousarray(
                np.broadcast_to(gamma.astype(np.float32), (128, HD))
            )
            m_map["beta_rep"] = np.ascontiguousarray(
                np.broadcast_to(beta.astype(np.float32), (128, HD))
            )
        in_maps.append(m_map)
    return in_maps


def kernel(x, adj, mask, W, a_src, a_dst, gamma, beta, _trace=False):
    from concourse.bass_utils import run_bass_kernel_spmd

    b, n, in_dim = x.shape
    ng = b // NCORES
    trivial_ln = bool(np.all(gamma == 1.0) and np.all(beta == 0.0))

    idxs = [np.nonzero(mask[g] > 0)[0] for g in range(b)]
    max_m = max((len(i) for i in idxs), default=1)
    NV = max(1, -(-max_m // 128))
    KC = in_dim // 128

    key = (ng, NV, KC, trivial_ln, repr(sorted(CFG.items())))
    if key not in _PROG_CACHE:
        _PROG_CACHE[key] = _build_program(ng, NV, KC, trivial_ln, CFG)
    nc = _PROG_CACHE[key]

    in_maps = _host_prep(
        x, adj, mask, W, a_src, a_dst, gamma, beta, ng, NV, idxs
    )
    res = run_bass_kernel_spmd(
        nc, in_maps, core_ids=list(range(NCORES)), trace=_trace
    )
    full = np.zeros((b, n, HD), np.float32)
    if not trivial_ln:
        full[:] = beta.astype(np.float32)[None, None, :]
    for c in range(NCORES):
        o = res.results[c]["out"].reshape(ng, NV * 128, HD)
        for gl in range(ng):
            g = c * ng + gl
            idx = idxs[g]
            full[g, idx] = o[gl, : len(idx)].astype(np.float32)
    if _trace:
        return full, res
    return full
